# revision 28
# baseline (speedup 1.0000x reference)
"""Distributed attention kernel for 8 TRN2 NeuronCores.

Problem: L=2048, B=2, E=256, H=8 heads, D=32 head-dim, fp32.

Sharding: DP2 over batch x sequence-parallel-4 over query positions.
Core c handles batch c//4, query rows [512*(c%4), 512*(c%4+1)), ALL 8
heads. k/v projections are redundantly computed per batch group (cheap)
and NO collective is needed: each core owns a disjoint output block.

The kernel is ScalarE-bound: softmax needs exp of H*TQ*L = 8.4M score
elements per core, and Exp only runs on ScalarE at 1 elem/lane/cycle
@1.2GHz (~55us/core floor + per-instruction overhead). The design goal
is a saturated ScalarE stream of [128, 1024] Exp instructions reading
PSUM directly, with ALL other work (PE matmuls, DVE bias/normalize,
DMA) hidden underneath.

Per-core pipeline (2 passes of 4 heads; heads 4p..4p+3 live at
partition bases 32u of kTp[p]/qTp[p]):
  scores: S.T[tk,tq] = kT_u^T qT_u as FOUR CONCURRENT row-tiled K=32
          matmuls (tile_position=(32u,0) auto-derived from base
          partitions) into two [128,1024] psum tiles (2 heads each),
          double-buffered.
  exp:    one [128,1024] Exp per 2 heads, f32 psum -> bf16 SBUF, with
          the 1/sqrt(D) scale fused.
  PV:     v STATIONARY ([v_h | 1] slots, 33 cols), P.T moving -- out is
          O.T[d,tq] with the softmax denominator Z.T as row 32,
          col-tiled 2x (head pair at psum partition bases 0/64 of one
          bank), accumulated over all 16 tk chunks in psum. This avoids
          the LDWEIGHTS-bound P-stationary form AND the output DMA
          transposes: O.T is exactly what the out-projection wants.
  norm:   DVE reciprocal of Z.T row, 32-lane stream_shuffle broadcast,
          tensor_tensor multiply -> oT [128(4 heads x 32d), 512] bf16.
  proj:   oT chunks stationary x Wp rows of this pass; pass partials
          accumulated in SBUF by DVE (psum banks are all busy:
          4 scores + 2 PV + 2 kqv/proj scratch).
"""

import os
import sys

import numpy as np

for _p in ("/opt/trn_rl_repo",):
    if _p not in sys.path and os.path.isdir(_p):
        sys.path.insert(0, _p)

import ml_dtypes

import concourse.bass as bass
import concourse.bacc as bacc
import concourse.mybir as mybir
import concourse.tile as tile
from concourse.bass_utils import run_bass_kernel_spmd

dt = mybir.dt
F32 = dt.float32
BF16 = dt.bfloat16
AF = mybir.ActivationFunctionType
ALU = mybir.AluOpType
BF = ml_dtypes.bfloat16

L, B, E, H, D = 2048, 2, 256, 8, 32
SCALE = float(D) ** -0.5
NCORES = 8
SP = 4            # sequence-parallel ways
TQ = L // SP      # 512 query rows per core
NTK = L // 128    # 16 tk chunks
VW = H * (D + 1)  # v_buf cols per tk chunk: 8x [v_h | 1] = 264
NPASS = 2         # head passes (4 heads each)

_GRAPH = None


def _build_graph():
    nc = bacc.Bacc(
        "TRN2",
        target_bir_lowering=False,
        debug=False,
        enable_asserts=False,
        num_devices=NCORES,
    )

    xqt = nc.declare_dram_parameter("xqt", [E, TQ], BF16, isOutput=False).ap()
    xkt = nc.declare_dram_parameter("xkt", [E, L], BF16, isOutput=False).ap()
    xvt = nc.declare_dram_parameter("xvt", [E, L], BF16, isOutput=False).ap()
    wq = nc.declare_dram_parameter("wq", [E, E], BF16, isOutput=False).ap()
    wk = nc.declare_dram_parameter("wk", [E, E], BF16, isOutput=False).ap()
    wv = nc.declare_dram_parameter("wv", [E, E], BF16, isOutput=False).ap()
    wp = nc.declare_dram_parameter("wp", [E, E], BF16, isOutput=False).ap()
    bq = nc.declare_dram_parameter("bq", [1, E], F32, isOutput=False).ap()
    bk = nc.declare_dram_parameter("bk", [1, E], F32, isOutput=False).ap()
    bv = nc.declare_dram_parameter("bv", [1, E], F32, isOutput=False).ap()
    bp = nc.declare_dram_parameter("bp", [1, E], F32, isOutput=False).ap()
    out = nc.declare_dram_parameter("out", [TQ, E], F32, isOutput=True).ap()

    with tile.TileContext(nc) as tc:
        with (
            tc.tile_pool(name="persist", bufs=1) as pp,
            tc.tile_pool(name="pt", bufs=10) as ptp,
            tc.tile_pool(name="osb", bufs=2) as osbp,
            tc.tile_pool(name="outsb", bufs=4) as outp,
            tc.tile_pool(name="st", bufs=2, space="PSUM") as stp,
            tc.tile_pool(name="po", bufs=2, space="PSUM") as pop,
            tc.tile_pool(name="ps", bufs=2, space="PSUM") as psp,
        ):
            # ---------- phase 0: warm + loads ----------
            # PE HAM warmup: ~3.4us of back-to-back dummy matmuls fill
            # the initial DMA window. The HAM clock gate only lifts
            # (1.2 -> 2.4 GHz) after ~3.4us of SUSTAINED PE activity,
            # and the steady-state chunk loop never has a burst that
            # long -- without this, every matmul in the kernel runs at
            # half clock (measured throttle_active ~71us of a 123us
            # span). The wsc memset is the FIRST DVE instruction so the
            # burst starts as early as possible and ends right as the
            # first projection inputs land.
            wsc = pp.tile([128, 512], BF16)
            nc.vector.memset(wsc[:], 0.0)
            wps = psp.tile([128, 512], F32, tag="ps", name="wps")
            for _ in range(10):
                nc.tensor.matmul(
                    wps[:], wsc[:, 0:128], wsc[:],
                    start=True, stop=True,
                )

            # weights: tile [128, 2E]; slice e covers W rows [128e, ..)
            # on the ScalarE HWDGE queue -- issued before the exp stream
            # starts (only SP/ACT have HWDGE; SyncE carries the x.T
            # stream). wk/wq first (critical path to the first score),
            # then the ACT table-load warm exp (~2.7us, overlaps the
            # k/q projections), then wv/wp.
            w_sb = {}

            def load_w(name, wsrc):
                t = pp.tile([128, 2 * E], BF16, name=f"w{name}", tag=f"w{name}")
                nc.scalar.dma_start(
                    out=t[:].rearrange("p (e n) -> p e n", e=2),
                    in_=wsrc.rearrange("(e p) n -> p e n", p=128),
                )
                w_sb[name] = t

            load_w("k", wk)
            load_w("q", wq)
            warm = pp.tile([1, 16], F32)
            nc.vector.memset(warm[:], 0.0)
            nc.scalar.activation(warm[:], warm[:], AF.Exp)
            load_w("v", wv)
            load_w("p", wp)

            # biases on gpsimd SWDGE: the bq/bk per-partition gathers are
            # 128x 4B descriptors -- latency-bound junk that must NOT sit
            # at the head of the SyncE queue in front of the x.T stream.
            bk_sb = pp.tile([128, 2], F32)
            nc.gpsimd.dma_start(
                out=bk_sb[:], in_=bk.rearrange("a (c p) -> p (a c)", p=128)
            )
            bq_sb = pp.tile([128, 2], F32)
            nc.gpsimd.dma_start(
                out=bq_sb[:], in_=bq.rearrange("a (c p) -> p (a c)", p=128)
            )
            bv_sb = pp.tile([128, E], F32)
            nc.gpsimd.dma_start(out=bv_sb[:], in_=bv.to_broadcast((128, E)))
            bp_sb = pp.tile([128, E], F32)
            nc.gpsimd.dma_start(out=bp_sb[:], in_=bp.to_broadcast((128, E)))

            # x.T streams on the SyncE HWDGE queue. DMA issue costs
            # ~0.6us of SP time EACH and all in-flight transfers share
            # ~300GB/s of HBM, so: few transfers, critical-first.
            # First 512-chunks of xk/xq/xv unblock the first projections;
            # the remaining 3/4 land as one big transfer per tile.
            xk_sb = [
                pp.tile([128, L], BF16, name=f"xkt{e}", tag=f"xkt{e}")
                for e in range(2)
            ]
            xv_sb = [
                pp.tile([128, L], BF16, name=f"xvt{e}", tag=f"xvt{e}")
                for e in range(2)
            ]
            xq_sb = [
                pp.tile([128, TQ], BF16, name=f"xqt{e}", tag=f"xqt{e}")
                for e in range(2)
            ]

            def load_x(dst, src, e, lo, hi):
                nc.sync.dma_start(
                    out=dst[e][:, lo:hi],
                    in_=src[e * 128:(e + 1) * 128, lo:hi],
                )

            for e in range(2):
                load_x(xk_sb, xkt, e, 0, 512)
            for e in range(2):
                nc.sync.dma_start(
                    out=xq_sb[e][:], in_=xqt[e * 128:(e + 1) * 128, :]
                )
            for e in range(2):
                load_x(xv_sb, xvt, e, 0, 512)
            for e in range(2):
                load_x(xk_sb, xkt, e, 512, L)
            for e in range(2):
                load_x(xv_sb, xvt, e, 512, L)

            # v_buf: per tk chunk, 8x [v_h (32) | 1] slots. Only the
            # ones-columns need initialization; v slots are fully
            # written by the vproj bias add.
            v_buf = pp.tile([128, NTK * VW], BF16)
            v_r = v_buf[:].rearrange("p (t h w) -> p t h w", t=NTK, h=H)
            nc.vector.memset(v_r[:, :, :, D:D + 1], 1.0)

            # kTp[p]/qTp[p]: pass p holds heads 4p..4p+3, head u=h-4p at
            # partitions [32u, 32u+32) -- the row-tiled score matmuls
            # and the col-tiled PV/proj all want exactly this layout.
            kTp = [pp.tile([128, L], BF16, name=f"kTp{p}", tag=f"kTp{p}")
                   for p in range(NPASS)]
            qTp = [pp.tile([128, TQ], BF16, name=f"qTp{p}", tag=f"qTp{p}")
                   for p in range(NPASS)]

            def kproj(hc, n):
                ps = psp.tile([128, 512], F32, tag="ps")
                for e in range(2):
                    nc.tensor.matmul(
                        ps[:],
                        w_sb["k"][:, e * E + hc * 128: e * E + (hc + 1) * 128],
                        xk_sb[e][:, n * 512:(n + 1) * 512],
                        start=(e == 0),
                        stop=(e == 1),
                    )
                nc.vector.tensor_scalar_add(
                    kTp[hc][:, n * 512:(n + 1) * 512], ps[:],
                    bk_sb[:, hc:hc + 1],
                )

            def qproj(hc):
                ps = psp.tile([128, 512], F32, tag="ps")
                for e in range(2):
                    nc.tensor.matmul(
                        ps[:],
                        w_sb["q"][:, e * E + hc * 128: e * E + (hc + 1) * 128],
                        xq_sb[e][:],
                        start=(e == 0),
                        stop=(e == 1),
                    )
                nc.vector.tensor_scalar_add(
                    qTp[hc][:], ps[:], bq_sb[:, hc:hc + 1],
                )

            def vproj(t):
                # v natural [tk 128, 256]: bias-added straight into the
                # strided [v_h | 1] slots of v_buf (bf16).
                ps = psp.tile([128, E], F32, tag="ps")
                for e in range(2):
                    nc.tensor.matmul(
                        ps[:],
                        xv_sb[e][:, t * 128:(t + 1) * 128],
                        w_sb["v"][:, e * E:(e + 1) * E],
                        start=(e == 0),
                        stop=(e == 1),
                    )
                nc.vector.tensor_tensor(
                    v_r[:, t, :, 0:D],
                    ps[:].rearrange("p (h d) -> p h d", h=H),
                    bv_sb[:].rearrange("p (h d) -> p h d", h=H),
                    ALU.add,
                )

            # SBUF accumulators for the output projection (psum is full).
            acc = [pp.tile([128, E], F32, name=f"acc{m}", tag=f"acc{m}")
                   for m in range(4)]
            # normalization scratch: Z rows of all 4 heads of a pass are
            # gathered into zs at partitions 32u, reciprocal'd in ONE
            # instruction (DVE reciprocal is free-size-bound: [128,512]
            # costs the same 3.3us as [1,512]), then one stream_shuffle
            # broadcasts partition 32u across each 32-partition group.
            # memset: the gather only writes 4 of 128 partitions.
            zs = pp.tile([128, 512], F32)
            nc.vector.memset(zs[:], 1.0)
            rs = pp.tile([128, 512], F32)
            rzb = pp.tile([128, 512], F32)

            # deferred projection-work queue: kq projections for pass 1
            # and v projections are interleaved into pass 0's chunk loop
            # to keep the PE fed while exps stream.
            pe_filler = []
            pe_filler.append(lambda: kproj(0, 1))
            pe_filler.append(lambda: vproj(2))
            pe_filler.append(lambda: vproj(3))
            pe_filler.append(lambda: kproj(0, 2))
            pe_filler.append(lambda: kproj(0, 3))
            pe_filler.append(lambda: kproj(1, 0))
            pe_filler.append(lambda: qproj(1))
            for t in range(4, NTK):
                pe_filler.append(lambda t=t: vproj(t))
            pe_filler.append(lambda: kproj(1, 1))
            pe_filler.append(lambda: kproj(1, 2))
            pe_filler.append(lambda: kproj(1, 3))
            fill_i = 0

            def fill(k):
                nonlocal fill_i
                for _ in range(k):
                    if fill_i < len(pe_filler):
                        pe_filler[fill_i]()
                        fill_i += 1

            # minimal prologue: just what S(p0, t=0) needs, then start
            # the exp stream. vproj(0)/(1) are emitted right after the
            # first scores: their xv dependency must not head-of-line
            # block the first score matmuls.
            kproj(0, 0)
            qproj(0)

            def scores_r(p, t, r):
                """2 concurrent row-tiled K=32 matmuls for head pair r
                (heads 2r, 2r+1) of chunk t -> one [128,1024] psum tile."""
                st = stp.tile([128, 1024], F32, tag="st", name=f"st{r}")
                for j in range(2):
                    u = 2 * r + j
                    nc.tensor.matmul(
                        st[:, j * 512:(j + 1) * 512],
                        kTp[p][32 * u:32 * (u + 1), t * 128:(t + 1) * 128],
                        qTp[p][32 * u:32 * (u + 1), :],
                        start=True,
                        stop=True,
                        # base partition 96 trips the auto-derive
                        # assert; pass the row tile explicitly
                        tile_position=(32 * u, 0),
                    )
                return st

            def exp_r(st):
                pt = ptp.tile([128, 1024], BF16, tag="pt")
                nc.scalar.activation(pt[:], st[:], AF.Exp, scale=SCALE)
                return pt

            def pv_r(p, t, po, pt, r):
                # v stationary [v|1] (33 cols), P.T moving: col-tiled
                # head pair per bank at psum partition bases 0 / 64.
                for j, base in ((0, 0), (1, 64)):
                    h = 4 * p + 2 * r + j
                    nc.tensor.matmul(
                        po[r][base:base + 33, :],
                        v_buf[:, t * VW + h * 33: t * VW + h * 33 + 33],
                        pt[:, j * 512:(j + 1) * 512],
                        start=(t == 0),
                        stop=(t == NTK - 1),
                        # two col-tiled groups share the bank (bases
                        # 0/64); pending-zero is partition-scoped so
                        # this is safe -- the sim's group check is
                        # bank-wide conservative.
                        skip_group_check=True,
                    )

            def normalize(p, po):
                # In the kernel tail (p=1) ScalarE is idle, so half the
                # Z-row gathers run there, shortening the serial DVE
                # chain. At the pass-0 boundary ScalarE must keep
                # streaming exps, so everything stays on DVE.
                osb = osbp.tile([128, TQ], BF16, tag="osb")
                for r in range(2):
                    for j, base in ((0, 0), (1, 64)):
                        u = 2 * r + j
                        if p == 1 and r == 0:
                            nc.scalar.copy(
                                zs[32 * u:32 * u + 1, :],
                                po[r][base + D:base + D + 1, :],
                            )
                        else:
                            nc.vector.tensor_scalar_add(
                                zs[32 * u:32 * u + 1, :],
                                po[r][base + D:base + D + 1, :],
                                0.0,
                            )
                nc.vector.reciprocal(rs[:], zs[:])
                nc.vector.stream_shuffle(rzb[:], rs[:], [0] * 32)
                for r in range(2):
                    for j, base in ((0, 0), (1, 64)):
                        u = 2 * r + j
                        nc.vector.tensor_tensor(
                            osb[32 * u:32 * (u + 1), :],
                            po[r][base:base + D, :],
                            rzb[32 * u:32 * (u + 1), :],
                            ALU.mult,
                        )
                return osb

            def proj(p, osb):
                for m in range(4):
                    ps = psp.tile([128, E], F32, tag="ps")
                    nc.tensor.matmul(
                        ps[:],
                        osb[:, m * 128:(m + 1) * 128],
                        w_sb["p"][:, p * E:(p + 1) * E],
                        start=True,
                        stop=True,
                    )
                    if p == 0:
                        # fold the output bias in now; pass 1 adds acc.
                        nc.vector.tensor_tensor(
                            acc[m][:], ps[:], bp_sb[:], ALU.add
                        )
                    else:
                        ob = outp.tile([128, E], F32, tag="outsb")
                        nc.vector.tensor_tensor(
                            ob[:], ps[:], acc[m][:], ALU.add
                        )
                        eng = nc.sync if m % 2 == 0 else nc.scalar
                        eng.dma_start(
                            out=out[m * 128:(m + 1) * 128, :], in_=ob[:]
                        )

            # ---------- phase 2: the exp-saturated main loop ----------
            # Software-pipelined: scores for chunk t+1 are emitted (and
            # run on the PE) while ScalarE exps chunk t, so the exp
            # stream never waits on a just-issued matmul. The PE filler
            # (pass-1 kq projections, v projections) rides in the gaps.
            # PE emission per chunk is phased by exp completions: after
            # exp(t,r) retires, S(t+1,r) (frees into its slot) and
            # PV(t,r) (consumes pt(t,r)) are both runnable -- pairing
            # them keeps the PE bursts dense so the HAM clock gate stays
            # open.
            po0 = [pop.tile([128, 512], F32, tag="po", name=f"po0_{r}")
                   for r in range(2)]
            pts = [exp_r(scores_r(0, 0, 0)), exp_r(scores_r(0, 0, 1))]
            vproj(0)
            vproj(1)
            for t in range(NTK):
                pn, tn = (0, t + 1) if t < NTK - 1 else (1, 0)
                nxt = []
                for r in range(2):
                    nxt.append(scores_r(pn, tn, r))
                    pv_r(0, t, po0, pts[r], r)
                    fill(1)
                pts = [exp_r(nxt[0]), exp_r(nxt[1])]
            fill(len(pe_filler))
            osb0 = normalize(0, po0)

            po1 = [pop.tile([128, 512], F32, tag="po", name=f"po1_{r}")
                   for r in range(2)]
            # Deep lookahead across the pass boundary: pv(1,0) WAR-waits
            # on normalize(0) freeing the po banks (~6us of DVE), and the
            # PE stream is in-order, so 4 chunks of scores (and their
            # exps) are emitted ahead of it to keep ScalarE streaming.
            pts_q = [pts]
            for t in range(1, 5):
                pts_q.append([exp_r(scores_r(1, t, 0)),
                              exp_r(scores_r(1, t, 1))])
            for t in range(NTK):
                cur = pts_q.pop(0)
                nxt = [] if t + 5 < NTK else None
                for r in range(2):
                    if nxt is not None:
                        nxt.append(scores_r(1, t + 5, r))
                    pv_r(1, t, po1, cur[r], r)
                if t == 5:
                    # pass-0 epilogue overlaps pass 1's exp stream (osb0
                    # is ready by now, so it can't head-of-line-block PE)
                    proj(0, osb0)
                if nxt is not None:
                    pts_q.append([exp_r(nxt[0]), exp_r(nxt[1])])
            osb1 = normalize(1, po1)
            proj(1, osb1)

    return nc


def get_graph():
    global _GRAPH
    if _GRAPH is None:
        nc = _build_graph()
        nc.compile()
        _GRAPH = nc
    return _GRAPH


def make_in_maps(query, key_, value, Wq, bq, Wk, bk, Wv, bv, Wp, bp):
    query = np.asarray(query, np.float32)
    key_ = np.asarray(key_, np.float32)
    value = np.asarray(value, np.float32)
    Wq, Wk, Wv, Wp = (np.asarray(w, np.float32) for w in (Wq, Wk, Wv, Wp))
    bq, bk, bv, bp = (np.asarray(b_, np.float32) for b_ in (bq, bk, bv, bp))

    wq_b = np.ascontiguousarray(Wq).astype(BF)
    wk_b = np.ascontiguousarray(Wk).astype(BF)
    wv_b = np.ascontiguousarray(Wv).astype(BF)
    wp_b = np.ascontiguousarray(Wp).astype(BF)
    xt = {}
    for b in range(B):
        xt[("q", b)] = np.ascontiguousarray(query[:, b, :].T).astype(BF)
        xt[("k", b)] = np.ascontiguousarray(key_[:, b, :].T).astype(BF)
        xt[("v", b)] = np.ascontiguousarray(value[:, b, :].T).astype(BF)

    in_maps = []
    for c in range(NCORES):
        b = c // SP
        p = c % SP
        m = {
            "xqt": np.ascontiguousarray(xt[("q", b)][:, p * TQ:(p + 1) * TQ]),
            "xkt": xt[("k", b)],
            "xvt": xt[("v", b)],
            "wq": wq_b,
            "wk": wk_b,
            "wv": wv_b,
            "wp": wp_b,
            "bq": bq.reshape(1, E).copy(),
            "bk": bk.reshape(1, E).copy(),
            "bv": bv.reshape(1, E).copy(),
            "bp": bp.reshape(1, E).copy(),
        }
        in_maps.append(m)
    return in_maps


def assemble(results):
    out_full = np.empty((L, B, E), np.float32)
    for c in range(NCORES):
        b = c // SP
        p = c % SP
        out_full[p * TQ:(p + 1) * TQ, b, :] = results[c]["out"]
    return out_full


def run(inputs, trace=False, **kw):
    nc = get_graph()
    in_maps = make_in_maps(**inputs)
    res = run_bass_kernel_spmd(
        nc, in_maps, core_ids=list(range(NCORES)), trace=trace, **kw
    )
    return res


def kernel(**inputs):
    res = run(inputs, trace=False)
    return assemble(res.results)
